# revision 2
# baseline (speedup 1.0000x reference)
"""GRU-D cell on 8 NeuronCores via a Bass/Tile kernel.

Data-parallel: batch dim 16384 is sharded 8 x 2048 across cores; the six
512x512 weight matrices plus per-feature vectors are replicated. All
device compute runs in a transposed [feature, batch] layout so the
contraction dim sits on SBUF partitions:

  - host casts inputs to bf16; device does DMA-transpose loads
  - gamma_x/gamma_h = exp(-relu(decay) * dt) on the ACT engine with the
    per-feature -relu(decay) as a per-partition scale operand
  - x_decayed / h_decayed imputation math on the vector engine
  - z/r/h_hat pre-activations accumulate in PSUM over 8 matmuls each
    (W.T @ x_decayed^T and U.T @ {h_decayed,r*h_decayed}^T, bf16)
  - sigmoid is computed as 0.5 + 0.5*tanh(x/2) so only the exp/tanh ACT
    table set is ever loaded (no ~2.7us table switches)
  - h_new stored transposed in bf16; host untransposes and casts to f32
"""

import numpy as np
import ml_dtypes

F = 512          # feature dim == units
B = 16384        # full batch
N_CORES = 8
BC = B // N_CORES     # per-core batch rows (2048)
NB = 512              # batch-column chunk (matmul free dim)
NCHUNK = BC // NB     # chunks per core (4)
KT = F // 128         # feature tiles (4)

BF16 = ml_dtypes.bfloat16

_STATE = {}


def _build():
    import concourse.bass as bass
    import concourse.mybir as mybir
    from concourse import bacc
    from concourse.tile import TileContext

    dt = mybir.dt
    AF = mybir.ActivationFunctionType
    OP = mybir.AluOpType

    nc = bacc.Bacc("TRN2", num_devices=N_CORES, debug=False)

    x_d = nc.dram_tensor("x", [BC, F], dt.bfloat16, kind="ExternalInput").ap()
    m_d = nc.dram_tensor("m", [BC, F], dt.bfloat16, kind="ExternalInput").ap()
    d_d = nc.dram_tensor("d", [BC, F], dt.bfloat16, kind="ExternalInput").ap()
    h_d = nc.dram_tensor("h", [BC, F], dt.bfloat16, kind="ExternalInput").ap()
    w_d = nc.dram_tensor("w", [F, 3 * F], dt.bfloat16, kind="ExternalInput").ap()
    u_d = nc.dram_tensor("u", [F, 3 * F], dt.bfloat16, kind="ExternalInput").ap()
    c_d = nc.dram_tensor("c", [128, KT, 6], dt.float32, kind="ExternalInput").ap()
    o_d = nc.dram_tensor("o", [F, BC], dt.bfloat16, kind="ExternalOutput").ap()

    # const column indices in c_d
    NGX, NGH, MI, BZ, BR, BH = range(6)

    with TileContext(nc) as tc:
        with (
            tc.tile_pool(name="const", bufs=1) as cpool,
            tc.tile_pool(name="io", bufs=2) as io,
            tc.tile_pool(name="work", bufs=2) as wk,
            tc.tile_pool(name="tmp", bufs=3) as tp,
            tc.tile_pool(name="psum", bufs=8, space="PSUM") as pp,
        ):
            # ---- resident weights / consts ----
            wsb = cpool.tile([128, KT, 3 * F], dt.bfloat16, tag="wsb")
            nc.sync.dma_start(out=wsb[:], in_=w_d.rearrange("(kt p) n -> p kt n", p=128))
            usb = cpool.tile([128, KT, 3 * F], dt.bfloat16, tag="usb")
            nc.sync.dma_start(out=usb[:], in_=u_d.rearrange("(kt p) n -> p kt n", p=128))
            csb = cpool.tile([128, KT, 6], dt.float32, tag="csb")
            nc.sync.dma_start(out=csb[:], in_=c_d)

            o_re = o_d.rearrange("(ut p) b -> p ut b", p=128)

            for c in range(NCHUNK):
                rows = slice(c * NB, (c + 1) * NB)

                # ---- transposed loads: [128 feat, NB batch] per k-tile ----
                dT = io.tile([128, KT, NB], dt.bfloat16, tag="dT")
                hT = io.tile([128, KT, NB], dt.bfloat16, tag="hT")
                xT = io.tile([128, KT, NB], dt.bfloat16, tag="xT")
                mT = io.tile([128, KT, NB], dt.bfloat16, tag="mT")
                for t in range(KT):
                    cols = slice(t * 128, (t + 1) * 128)
                    nc.sync.dma_start_transpose(dT[:, t, :], d_d[rows, cols])
                    nc.sync.dma_start_transpose(hT[:, t, :], h_d[rows, cols])
                    nc.sync.dma_start_transpose(xT[:, t, :], x_d[rows, cols])
                    nc.sync.dma_start_transpose(mT[:, t, :], m_d[rows, cols])

                # ---- elementwise preprocessing ----
                hd = wk.tile([128, KT, NB], dt.bfloat16, tag="hd")
                xd = wk.tile([128, KT, NB], dt.bfloat16, tag="xd")
                for t in range(KT):
                    gh = tp.tile([128, NB], dt.bfloat16, tag="gh")
                    nc.scalar.activation(gh[:], dT[:, t, :], AF.Exp, scale=csb[:, t, NGH:NGH + 1])
                    nc.vector.tensor_mul(out=hd[:, t, :], in0=gh[:], in1=hT[:, t, :])

                    gx = tp.tile([128, NB], dt.bfloat16, tag="gx")
                    nc.scalar.activation(gx[:], dT[:, t, :], AF.Exp, scale=csb[:, t, NGX:NGX + 1])
                    # v = gx*(x - mi) + mi ; xd = v + m*(x - v)
                    t1 = tp.tile([128, NB], dt.bfloat16, tag="t1")
                    nc.vector.tensor_scalar(t1[:], xT[:, t, :], csb[:, t, MI:MI + 1], None, OP.subtract)
                    t2 = tp.tile([128, NB], dt.bfloat16, tag="t2")
                    nc.vector.tensor_mul(out=t2[:], in0=gx[:], in1=t1[:])
                    t3 = tp.tile([128, NB], dt.bfloat16, tag="t3")
                    nc.vector.tensor_scalar(t3[:], t2[:], csb[:, t, MI:MI + 1], None, OP.add)
                    t4 = tp.tile([128, NB], dt.bfloat16, tag="t4")
                    nc.vector.tensor_sub(out=t4[:], in0=xT[:, t, :], in1=t3[:])
                    t5 = tp.tile([128, NB], dt.bfloat16, tag="t5")
                    nc.vector.tensor_mul(out=t5[:], in0=mT[:, t, :], in1=t4[:])
                    nc.vector.tensor_add(out=xd[:, t, :], in0=t3[:], in1=t5[:])

                # ---- gates z and r: psum = U.T @ hd + W.T @ xd, then
                # sigmoid(p + b) = 0.5 + 0.5*tanh((p + b)/2) ----
                gate_sb = {}
                for g, (boff, tag) in enumerate(((BZ, "zt"), (BR, "rt"))):
                    gt = wk.tile([128, KT, NB], dt.bfloat16, tag=tag)
                    gate_sb[tag] = gt
                    for ut in range(KT):
                        colw = slice(g * F + ut * 128, g * F + (ut + 1) * 128)
                        ps = pp.tile([128, NB], dt.float32, tag="ps")
                        for t in range(KT):
                            nc.tensor.matmul(ps[:], usb[:, t, colw], hd[:, t, :],
                                             start=(t == 0), stop=False)
                        for t in range(KT):
                            nc.tensor.matmul(ps[:], wsb[:, t, colw], xd[:, t, :],
                                             start=False, stop=(t == KT - 1))
                        tau = tp.tile([128, NB], dt.bfloat16, tag="tau")
                        nc.scalar.activation(tau[:], ps[:], AF.Tanh,
                                             bias=csb[:, ut, boff:boff + 1], scale=0.5)
                        nc.vector.tensor_scalar(gt[:, ut, :], tau[:], 0.5, 0.5, OP.mult, OP.add)

                # rh = r * hd
                rh = wk.tile([128, KT, NB], dt.bfloat16, tag="rh")
                for t in range(KT):
                    nc.vector.tensor_mul(out=rh[:, t, :], in0=gate_sb["rt"][:, t, :], in1=hd[:, t, :])

                # ---- h_hat = tanh(W_h.T @ xd + U_h.T @ rh + b_h) ----
                hh = wk.tile([128, KT, NB], dt.bfloat16, tag="hh")
                for ut in range(KT):
                    colw = slice(2 * F + ut * 128, 2 * F + (ut + 1) * 128)
                    ps = pp.tile([128, NB], dt.float32, tag="ps")
                    for t in range(KT):
                        nc.tensor.matmul(ps[:], wsb[:, t, colw], xd[:, t, :],
                                         start=(t == 0), stop=False)
                    for t in range(KT):
                        nc.tensor.matmul(ps[:], usb[:, t, colw], rh[:, t, :],
                                         start=False, stop=(t == KT - 1))
                    nc.scalar.activation(hh[:, ut, :], ps[:], AF.Tanh,
                                         bias=csb[:, ut, BH:BH + 1])

                # ---- h_new = hd + z*(hh - hd) ----
                hn = wk.tile([128, KT, NB], dt.bfloat16, tag="hn")
                zt = gate_sb["zt"]
                for t in range(KT):
                    t6 = tp.tile([128, NB], dt.bfloat16, tag="t6")
                    nc.vector.tensor_sub(out=t6[:], in0=hh[:, t, :], in1=hd[:, t, :])
                    t7 = tp.tile([128, NB], dt.bfloat16, tag="t7")
                    nc.vector.tensor_mul(out=t7[:], in0=zt[:, t, :], in1=t6[:])
                    nc.vector.tensor_add(out=hn[:, t, :], in0=hd[:, t, :], in1=t7[:])

                nc.sync.dma_start(out=o_re[:, :, rows], in_=hn[:])

    nc.compile()
    return nc


def _get_nc():
    if "nc" not in _STATE:
        _STATE["nc"] = _build()
    return _STATE["nc"]


def kernel(**inputs) -> np.ndarray:
    from concourse import bass_utils

    nc = _get_nc()

    inp = np.asarray(inputs["inputs"], dtype=np.float32)
    x = inp[:, :F].astype(BF16)
    m = inp[:, F:2 * F].astype(BF16)
    d = inp[:, 2 * F:].astype(BF16)
    h = np.asarray(inputs["h_prev"], dtype=np.float32).astype(BF16)

    w = np.concatenate(
        [np.asarray(inputs["W_z"]), np.asarray(inputs["W_r"]), np.asarray(inputs["W_h"])],
        axis=1).astype(BF16)
    u = np.concatenate(
        [np.asarray(inputs["U_z"]), np.asarray(inputs["U_r"]), np.asarray(inputs["U_h"])],
        axis=1).astype(BF16)

    consts = np.stack(
        [
            -np.maximum(np.asarray(inputs["gamma_x_decay"], np.float32), 0.0),
            -np.maximum(np.asarray(inputs["gamma_h_decay"], np.float32), 0.0),
            np.asarray(inputs["mean_imputation"], np.float32),
            np.asarray(inputs["b_z"], np.float32),
            np.asarray(inputs["b_r"], np.float32),
            np.asarray(inputs["b_h"], np.float32),
        ],
        axis=-1,
    ).reshape(KT, 128, 6).transpose(1, 0, 2)
    consts = np.ascontiguousarray(consts)

    in_maps = []
    for c in range(N_CORES):
        rows = slice(c * BC, (c + 1) * BC)
        in_maps.append({
            "x": x[rows], "m": m[rows], "d": d[rows], "h": h[rows],
            "w": w, "u": u, "c": consts,
        })

    res = bass_utils.run_bass_kernel_spmd(
        nc, in_maps, core_ids=list(range(N_CORES)), **_STATE.get("run_kwargs", {})
    )
    _STATE["last_results"] = res

    out = np.empty((B, F), np.float32)
    for c in range(N_CORES):
        out[c * BC:(c + 1) * BC, :] = res.results[c]["o"].T.astype(np.float32)
    return out


# revision 3
# speedup vs baseline: 1.1716x; 1.1716x over previous
"""GRU-D cell on 8 NeuronCores via a Bass/Tile kernel.

Data-parallel: batch 16384 -> 8 x 2048; the 512x512 weights are
replicated. All device compute runs in a transposed [feature, batch]
layout so the matmul contraction dim sits on SBUF partitions; the host
pre-casts to bf16 and pre-transposes the four big per-element tensors
(x, m, delta_t, h_prev) so every device DMA is a plain contiguous copy.

Per 512-column batch chunk on each core:
  gx    = exp(-relu(gamma)*dt)                  (ACT, one merged op)
  hd    = gx*h                                  (DVE)
  xd    = x*(m + gx - m*gx)                     (DVE, 3 fused ops; general
                                                 mean-imputation fallback)
  z',r' pre-acts accumulate over 8 matmuls each into one 4-bank PSUM tile
  tau   = tanh(pre/2)  [sigmoid via tanh, so only one ACT table set]
  rh'   = (tau_r+1)*hd   with U_h host-scaled by 0.5
  h_hat = tanh(W_h xd + U_h' rh')
  h_new = hd + (0.5*tau_z+0.5)*(h_hat-hd)       (DVE fused)

The program is built lazily per specialization (uniform gamma decays /
zero mean-imputation / zero biases -> merged single-instruction forms;
otherwise a general per-tile path with per-partition scale/bias APs).
"""

import numpy as np
import ml_dtypes

F = 512          # feature dim == units
B = 16384        # full batch
N_CORES = 8
BC = B // N_CORES     # per-core batch rows (2048)
NB = 512              # batch-column chunk (matmul free dim)
NCHUNK = BC // NB     # chunks per core (4)
KT = F // 128         # feature tiles (4)

BF16 = ml_dtypes.bfloat16

_STATE = {}


def _build(spec_key):
    """spec_key: ("spec", gx_scale) for the specialized program, or "gen"."""
    import concourse.mybir as mybir
    from concourse import bacc
    from concourse.tile import TileContext

    dt = mybir.dt
    AF = mybir.ActivationFunctionType
    OP = mybir.AluOpType

    specialized = spec_key[0] == "spec"

    nc = bacc.Bacc("TRN2", num_devices=N_CORES, debug=False)

    x_d = nc.dram_tensor("x", [F, BC], dt.bfloat16, kind="ExternalInput").ap()
    m_d = nc.dram_tensor("m", [F, BC], dt.bfloat16, kind="ExternalInput").ap()
    d_d = nc.dram_tensor("d", [F, BC], dt.bfloat16, kind="ExternalInput").ap()
    h_d = nc.dram_tensor("h", [F, BC], dt.bfloat16, kind="ExternalInput").ap()
    w_d = nc.dram_tensor("w", [F, 3 * F], dt.bfloat16, kind="ExternalInput").ap()
    u_d = nc.dram_tensor("u", [F, 3 * F], dt.bfloat16, kind="ExternalInput").ap()
    if not specialized:
        c_d = nc.dram_tensor("c", [128, KT, 6], dt.float32, kind="ExternalInput").ap()
    o_d = nc.dram_tensor("o", [F, BC], dt.bfloat16, kind="ExternalOutput").ap()

    NGX, NGH, MI, BZ, BR, BH = range(6)

    x_r = x_d.rearrange("(kt p) b -> p kt b", p=128)
    m_r = m_d.rearrange("(kt p) b -> p kt b", p=128)
    d_r = d_d.rearrange("(kt p) b -> p kt b", p=128)
    h_r = h_d.rearrange("(kt p) b -> p kt b", p=128)
    w_r = w_d.rearrange("(kt p) n -> p kt n", p=128)
    u_r = u_d.rearrange("(kt p) n -> p kt n", p=128)
    o_r = o_d.rearrange("(ut p) b -> p ut b", p=128)

    with TileContext(nc) as tc:
        with (
            tc.tile_pool(name="const", bufs=1) as cpool,
            tc.tile_pool(name="io", bufs=2) as io,
            tc.tile_pool(name="work", bufs=2) as wk,
            tc.tile_pool(name="tmp", bufs=3) as tp,
            tc.tile_pool(name="psum", bufs=2, space="PSUM") as pp,
        ):
            wsb = cpool.tile([128, KT, 3 * F], dt.bfloat16, tag="wsb")
            usb = cpool.tile([128, KT, 3 * F], dt.bfloat16, tag="usb")
            if not specialized:
                csb = cpool.tile([128, KT, 6], dt.float32, tag="csb")
                nc.sync.dma_start(out=csb[:], in_=c_d)
            # stage weight loads in need order: U_z, W_z, U_r, W_r, W_h, U_h
            for lo, hi, dst, src in (
                (0, F, usb, u_r), (0, F, wsb, w_r),
                (F, 2 * F, usb, u_r), (F, 2 * F, wsb, w_r),
                (2 * F, 3 * F, wsb, w_r), (2 * F, 3 * F, usb, u_r),
            ):
                nc.sync.dma_start(out=dst[:, :, lo:hi], in_=src[:, :, lo:hi])

            for c in range(NCHUNK):
                rows = slice(c * NB, (c + 1) * NB)
                last = c == NCHUNK - 1

                dT = io.tile([128, KT, NB], dt.bfloat16, tag="dT")
                hT = io.tile([128, KT, NB], dt.bfloat16, tag="hT")
                xT = io.tile([128, KT, NB], dt.bfloat16, tag="xT")
                mT = io.tile([128, KT, NB], dt.bfloat16, tag="mT")
                nc.sync.dma_start(out=dT[:], in_=d_r[:, :, rows])
                nc.sync.dma_start(out=hT[:], in_=h_r[:, :, rows])
                nc.sync.dma_start(out=xT[:], in_=x_r[:, :, rows])
                nc.sync.dma_start(out=mT[:], in_=m_r[:, :, rows])

                hd = wk.tile([128, KT, NB], dt.bfloat16, tag="hd")
                xd = wk.tile([128, KT, NB], dt.bfloat16, tag="xd")
                if specialized:
                    gx_scale = spec_key[1]
                    # gx = exp(gx_scale * dt), one merged ACT; gh == gx
                    gx = tp.tile([128, KT, NB], dt.bfloat16, tag="gx")
                    nc.scalar.activation(gx[:], dT[:], AF.Exp, scale=gx_scale)
                    nc.vector.tensor_mul(out=hd[:], in0=gx[:], in1=hT[:])
                    # xd = x*(m + gx - m*gx):  bm = (gx-1)*m ; s = gx-bm ; xd = x*s
                    bm = tp.tile([128, KT, NB], dt.bfloat16, tag="bm")
                    nc.vector.scalar_tensor_tensor(
                        bm[:], gx[:], 1.0, mT[:], OP.subtract, OP.mult)
                    s = tp.tile([128, KT, NB], dt.bfloat16, tag="s")
                    nc.vector.tensor_sub(out=s[:], in0=gx[:], in1=bm[:])
                    nc.vector.tensor_mul(out=xd[:], in0=xT[:], in1=s[:])
                else:
                    for t in range(KT):
                        gh = tp.tile([128, NB], dt.bfloat16, tag="gh")
                        nc.scalar.activation(gh[:], dT[:, t, :], AF.Exp,
                                             scale=csb[:, t, NGH:NGH + 1])
                        nc.vector.tensor_mul(out=hd[:, t, :], in0=gh[:], in1=hT[:, t, :])
                        gxt = tp.tile([128, NB], dt.bfloat16, tag="gxt")
                        nc.scalar.activation(gxt[:], dT[:, t, :], AF.Exp,
                                             scale=csb[:, t, NGX:NGX + 1])
                        # v = gx*(x-mi)+mi ; xd = v + m*(x-v)
                        p1 = tp.tile([128, NB], dt.bfloat16, tag="p1")
                        nc.vector.scalar_tensor_tensor(
                            p1[:], xT[:, t, :], csb[:, t, MI:MI + 1], gxt[:],
                            OP.subtract, OP.mult)
                        v = tp.tile([128, NB], dt.bfloat16, tag="v")
                        nc.vector.tensor_scalar(v[:], p1[:], csb[:, t, MI:MI + 1],
                                                None, OP.add)
                        q = tp.tile([128, NB], dt.bfloat16, tag="q")
                        nc.vector.tensor_sub(out=q[:], in0=xT[:, t, :], in1=v[:])
                        r2 = tp.tile([128, NB], dt.bfloat16, tag="r2")
                        nc.vector.tensor_mul(out=r2[:], in0=mT[:, t, :], in1=q[:])
                        nc.vector.tensor_add(out=xd[:, t, :], in0=v[:], in1=r2[:])

                # ---- gates z, r: tau = tanh((pre + b)/2) ----
                taus = {}
                for g, (boff, tag) in enumerate(((BZ, "tau_z"), (BR, "tau_r"))):
                    ps = pp.tile([128, KT, NB], dt.float32, tag="ps")
                    for ut in range(KT):
                        colw = slice(g * F + ut * 128, g * F + (ut + 1) * 128)
                        for t in range(KT):
                            nc.tensor.matmul(ps[:, ut, :], usb[:, t, colw], hd[:, t, :],
                                             start=(t == 0), stop=False)
                        for t in range(KT):
                            nc.tensor.matmul(ps[:, ut, :], wsb[:, t, colw], xd[:, t, :],
                                             start=False, stop=(t == KT - 1))
                    tau = wk.tile([128, KT, NB], dt.bfloat16, tag=tag)
                    taus[tag] = tau
                    if specialized:
                        nc.scalar.activation(tau[:], ps[:], AF.Tanh, scale=0.5)
                    else:
                        for ut in range(KT):
                            nc.scalar.activation(tau[:, ut, :], ps[:, ut, :], AF.Tanh,
                                                 bias=csb[:, ut, boff:boff + 1],
                                                 scale=0.5)
                        # bias must be halved too: tanh((pre+b)/2) needs bias b/2.
                        # csb stores b/2 for BZ/BR (host packs half-biases).

                # rh' = (tau_r + 1) * hd   (U_h is host-scaled by 0.5)
                rh = wk.tile([128, KT, NB], dt.bfloat16, tag="rh")
                nc.vector.scalar_tensor_tensor(
                    rh[:], taus["tau_r"][:], 1.0, hd[:], OP.add, OP.mult)

                # ---- h_hat pre-act ----
                ps_h = pp.tile([128, KT, NB], dt.float32, tag="ps")
                for ut in range(KT):
                    colw = slice(2 * F + ut * 128, 2 * F + (ut + 1) * 128)
                    for t in range(KT):
                        nc.tensor.matmul(ps_h[:, ut, :], wsb[:, t, colw], xd[:, t, :],
                                         start=(t == 0), stop=False)
                    for t in range(KT):
                        nc.tensor.matmul(ps_h[:, ut, :], usb[:, t, colw], rh[:, t, :],
                                         start=False, stop=(t == KT - 1))

                # ---- h_hat = tanh(. + b_h); h_new = hd + (0.5 tau_z + 0.5)(hh - hd)
                tau_z = taus["tau_z"]
                if last:
                    # split per u-tile so the kernel tail pipelines
                    for ut in range(KT):
                        hh = tp.tile([128, NB], dt.bfloat16, tag="hh_l")
                        if specialized:
                            nc.scalar.activation(hh[:], ps_h[:, ut, :], AF.Tanh)
                        else:
                            nc.scalar.activation(hh[:], ps_h[:, ut, :], AF.Tanh,
                                                 bias=csb[:, ut, BH:BH + 1])
                        t6 = tp.tile([128, NB], dt.bfloat16, tag="t6_l")
                        nc.vector.tensor_sub(out=t6[:], in0=hh[:], in1=hd[:, ut, :])
                        t7 = tp.tile([128, NB], dt.bfloat16, tag="t7_l")
                        nc.vector.scalar_tensor_tensor(
                            t7[:], tau_z[:, ut, :], 1.0, t6[:], OP.add, OP.mult)
                        hn = tp.tile([128, NB], dt.bfloat16, tag="hn_l")
                        nc.vector.scalar_tensor_tensor(
                            hn[:], t7[:], 0.5, hd[:, ut, :], OP.mult, OP.add)
                        nc.sync.dma_start(out=o_r[:, ut, rows], in_=hn[:])
                else:
                    hh = wk.tile([128, KT, NB], dt.bfloat16, tag="hh")
                    if specialized:
                        nc.scalar.activation(hh[:], ps_h[:], AF.Tanh)
                    else:
                        for ut in range(KT):
                            nc.scalar.activation(hh[:, ut, :], ps_h[:, ut, :], AF.Tanh,
                                                 bias=csb[:, ut, BH:BH + 1])
                    t6 = tp.tile([128, KT, NB], dt.bfloat16, tag="t6")
                    nc.vector.tensor_sub(out=t6[:], in0=hh[:], in1=hd[:])
                    t7 = tp.tile([128, KT, NB], dt.bfloat16, tag="t7")
                    nc.vector.scalar_tensor_tensor(
                        t7[:], tau_z[:], 1.0, t6[:], OP.add, OP.mult)
                    hn = wk.tile([128, KT, NB], dt.bfloat16, tag="hn")
                    nc.vector.scalar_tensor_tensor(
                        hn[:], t7[:], 0.5, hd[:], OP.mult, OP.add)
                    nc.sync.dma_start(out=o_r[:, :, rows], in_=hn[:])

    nc.compile()
    return nc


def _get_nc(spec_key):
    if spec_key not in _STATE:
        _STATE[spec_key] = _build(spec_key)
    return _STATE[spec_key]


def _tp_cast(a):
    """[B, F] f32 view -> [F, B] contiguous bf16."""
    return np.ascontiguousarray(a.T).astype(BF16)


def kernel(**inputs) -> np.ndarray:
    from concourse import bass_utils

    inp = np.asarray(inputs["inputs"], dtype=np.float32)
    h_prev = np.asarray(inputs["h_prev"], dtype=np.float32)
    gx = np.maximum(np.asarray(inputs["gamma_x_decay"], np.float32), 0.0)
    gh = np.maximum(np.asarray(inputs["gamma_h_decay"], np.float32), 0.0)
    mi = np.asarray(inputs["mean_imputation"], np.float32)
    bz = np.asarray(inputs["b_z"], np.float32)
    br = np.asarray(inputs["b_r"], np.float32)
    bh = np.asarray(inputs["b_h"], np.float32)

    specialized = (
        np.all(gx == gx[0]) and np.all(gh == gx[0])
        and not np.any(mi) and not np.any(bz) and not np.any(br) and not np.any(bh)
    )
    spec_key = ("spec", float(-gx[0])) if specialized else "gen"
    nc = _get_nc(spec_key)

    xT = _tp_cast(inp[:, :F])
    mT = _tp_cast(inp[:, F:2 * F])
    dT = _tp_cast(inp[:, 2 * F:])
    hT = _tp_cast(h_prev)

    w = np.concatenate(
        [np.asarray(inputs["W_z"]), np.asarray(inputs["W_r"]), np.asarray(inputs["W_h"])],
        axis=1).astype(BF16)
    # fold the sigmoid(tanh) 0.5 rescale of r into U_h
    u = np.concatenate(
        [np.asarray(inputs["U_z"]), np.asarray(inputs["U_r"]),
         0.5 * np.asarray(inputs["U_h"])],
        axis=1).astype(BF16)

    in_maps = []
    for c in range(N_CORES):
        cols = slice(c * BC, (c + 1) * BC)
        im = {"x": xT[:, cols], "m": mT[:, cols], "d": dT[:, cols], "h": hT[:, cols],
              "w": w, "u": u}
        if not specialized:
            # half-biases for z/r (tanh((pre+b)/2) takes b/2 as the ACT bias)
            consts = np.stack([-gx, -gh, mi, 0.5 * bz, 0.5 * br, bh], axis=-1)
            im["c"] = np.ascontiguousarray(
                consts.reshape(KT, 128, 6).transpose(1, 0, 2))
        in_maps.append(im)

    res = bass_utils.run_bass_kernel_spmd(
        nc, in_maps, core_ids=list(range(N_CORES)), **_STATE.get("run_kwargs", {})
    )
    _STATE["last_results"] = res

    out = np.empty((B, F), np.float32)
    for c in range(N_CORES):
        out[c * BC:(c + 1) * BC, :] = res.results[c]["o"].T.astype(np.float32)
    return out


# revision 4
# speedup vs baseline: 1.3062x; 1.1149x over previous
"""GRU-D cell on 8 NeuronCores via a Bass/Tile kernel.

Data-parallel: batch 16384 -> 8 x 2048; the 512x512 weights are
replicated. All device compute runs in a transposed [feature, batch]
layout so the matmul contraction dim sits on SBUF partitions; the host
pre-casts to bf16 and pre-transposes the four big per-element tensors
(x, m, delta_t, h_prev) so every device DMA is a plain contiguous copy.

Per 512-column batch chunk on each core (software-pipelined across
chunks; ACT/DVE queues are FIFO so emission order is execution order):
  gx    = exp(-relu(gamma)*dt)                  (ACT, one merged op)
  hd    = gx*h                                  (DVE)
  xd    = x*(m + gx - m*gx)                     (DVE, 3 fused ops; general
                                                 mean-imputation fallback)
  z/r   pre-acts accumulate over 8 matmuls per u-tile into 1-bank PSUM
  tau   = tanh(pre/2)  [sigmoid via tanh: one ACT table set, no reloads]
  rh'   = (tau_r+1)*hd   with U_h host-scaled by 0.5
  h_hat = tanh(W_h xd + U_h' rh')
  h_new = hd + (0.5*tau_z+0.5)*(h_hat-hd)       (DVE fused)

Weights ride the GpSimd SWDGE queue so chunk loads on the Sync HWDGE
queue aren't serialized behind them. The program is built lazily per
specialization (uniform gamma decays / zero mean-imputation / zero
biases; general per-tile fallback otherwise).
"""

import numpy as np
import ml_dtypes

F = 512          # feature dim == units
B = 16384        # full batch
N_CORES = 8
BC = B // N_CORES     # per-core batch rows (2048)
NB = 512              # batch-column chunk (matmul free dim)
NCHUNK = BC // NB     # chunks per core (4)
KT = F // 128         # feature tiles (4)

BF16 = ml_dtypes.bfloat16

_STATE = {}


def _build(spec_key):
    """spec_key: ("spec", gx_scale) for the specialized program, or "gen"."""
    import concourse.mybir as mybir
    from concourse import bacc
    from concourse.tile import TileContext

    dt = mybir.dt
    AF = mybir.ActivationFunctionType
    OP = mybir.AluOpType

    specialized = spec_key[0] == "spec"

    nc = bacc.Bacc("TRN2", num_devices=N_CORES, debug=False)

    x_d = nc.dram_tensor("x", [F, BC], dt.bfloat16, kind="ExternalInput").ap()
    m_d = nc.dram_tensor("m", [F, BC], dt.bfloat16, kind="ExternalInput").ap()
    d_d = nc.dram_tensor("d", [F, BC], dt.bfloat16, kind="ExternalInput").ap()
    h_d = nc.dram_tensor("h", [F, BC], dt.bfloat16, kind="ExternalInput").ap()
    w_d = nc.dram_tensor("w", [F, 3 * F], dt.bfloat16, kind="ExternalInput").ap()
    u_d = nc.dram_tensor("u", [F, 3 * F], dt.bfloat16, kind="ExternalInput").ap()
    if not specialized:
        c_d = nc.dram_tensor("c", [128, KT, 6], dt.float32, kind="ExternalInput").ap()
    o_d = nc.dram_tensor("o", [F, BC], dt.bfloat16, kind="ExternalOutput").ap()

    NGX, NGH, MI, BZ, BR, BH = range(6)

    x_r = x_d.rearrange("(kt p) b -> p kt b", p=128)
    m_r = m_d.rearrange("(kt p) b -> p kt b", p=128)
    d_r = d_d.rearrange("(kt p) b -> p kt b", p=128)
    h_r = h_d.rearrange("(kt p) b -> p kt b", p=128)
    w_r = w_d.rearrange("(kt p) n -> p kt n", p=128)
    u_r = u_d.rearrange("(kt p) n -> p kt n", p=128)
    o_r = o_d.rearrange("(ut p) b -> p ut b", p=128)

    with TileContext(nc) as tc:
        with (
            tc.tile_pool(name="const", bufs=1) as cpool,
            tc.tile_pool(name="io", bufs=2) as io,
            tc.tile_pool(name="work", bufs=2) as wk,
            tc.tile_pool(name="tmp", bufs=3) as tp,
            tc.tile_pool(name="psum", bufs=8, space="PSUM") as pp,
        ):
            wsb = cpool.tile([128, KT, 3 * F], dt.bfloat16, tag="wsb")
            usb = cpool.tile([128, KT, 3 * F], dt.bfloat16, tag="usb")
            if not specialized:
                csb = cpool.tile([128, KT, 6], dt.float32, tag="csb")
                nc.sync.dma_start(out=csb[:], in_=c_d)

            def load_chunk(c):
                rows = slice(c * NB, (c + 1) * NB)
                dT = io.tile([128, KT, NB], dt.bfloat16, tag="dT")
                hT = io.tile([128, KT, NB], dt.bfloat16, tag="hT")
                xT = io.tile([128, KT, NB], dt.bfloat16, tag="xT")
                mT = io.tile([128, KT, NB], dt.bfloat16, tag="mT")
                nc.sync.dma_start(out=dT[:], in_=d_r[:, :, rows])
                nc.sync.dma_start(out=hT[:], in_=h_r[:, :, rows])
                nc.sync.dma_start(out=xT[:], in_=x_r[:, :, rows])
                nc.sync.dma_start(out=mT[:], in_=m_r[:, :, rows])
                return dT, hT, xT, mT

            def exp_chunk(tiles):
                dT, hT, xT, mT = tiles
                if specialized:
                    gx = tp.tile([128, KT, NB], dt.bfloat16, tag="gx")
                    nc.scalar.activation(gx[:], dT[:], AF.Exp, scale=spec_key[1])
                    return (gx,)
                gxs, ghs = [], []
                for t in range(KT):
                    gh = tp.tile([128, NB], dt.bfloat16, tag=f"gh{t}")
                    nc.scalar.activation(gh[:], dT[:, t, :], AF.Exp,
                                         scale=csb[:, t, NGH:NGH + 1])
                    gxt = tp.tile([128, NB], dt.bfloat16, tag=f"gxt{t}")
                    nc.scalar.activation(gxt[:], dT[:, t, :], AF.Exp,
                                         scale=csb[:, t, NGX:NGX + 1])
                    gxs.append(gxt); ghs.append(gh)
                return (gxs, ghs)

            def preproc_chunk(tiles, gtiles):
                dT, hT, xT, mT = tiles
                hd = wk.tile([128, KT, NB], dt.bfloat16, tag="hd")
                xd = wk.tile([128, KT, NB], dt.bfloat16, tag="xd")
                if specialized:
                    (gx,) = gtiles
                    nc.vector.tensor_mul(out=hd[:], in0=gx[:], in1=hT[:])
                    # xd = x*(m + gx - m*gx): bm=(gx-1)*m ; s=gx-bm ; xd=x*s
                    bm = tp.tile([128, KT, NB], dt.bfloat16, tag="bm")
                    nc.vector.scalar_tensor_tensor(
                        bm[:], gx[:], 1.0, mT[:], OP.subtract, OP.mult)
                    s = tp.tile([128, KT, NB], dt.bfloat16, tag="s")
                    nc.vector.tensor_sub(out=s[:], in0=gx[:], in1=bm[:])
                    nc.vector.tensor_mul(out=xd[:], in0=xT[:], in1=s[:])
                else:
                    gxs, ghs = gtiles
                    for t in range(KT):
                        nc.vector.tensor_mul(out=hd[:, t, :], in0=ghs[t][:],
                                             in1=hT[:, t, :])
                        # v = gx*(x-mi)+mi ; xd = v + m*(x-v)
                        p1 = tp.tile([128, NB], dt.bfloat16, tag="p1")
                        nc.vector.scalar_tensor_tensor(
                            p1[:], xT[:, t, :], csb[:, t, MI:MI + 1], gxs[t][:],
                            OP.subtract, OP.mult)
                        v = tp.tile([128, NB], dt.bfloat16, tag="v")
                        nc.vector.tensor_scalar(v[:], p1[:], csb[:, t, MI:MI + 1],
                                                None, OP.add)
                        q = tp.tile([128, NB], dt.bfloat16, tag="q")
                        nc.vector.tensor_sub(out=q[:], in0=xT[:, t, :], in1=v[:])
                        r2 = tp.tile([128, NB], dt.bfloat16, tag="r2")
                        nc.vector.tensor_mul(out=r2[:], in0=mT[:, t, :], in1=q[:])
                        nc.vector.tensor_add(out=xd[:, t, :], in0=v[:], in1=r2[:])
                return hd, xd

            def gate_mms(base, lhs_a, rhs_a, lhs_b, rhs_b):
                """8 accumulating MMs per u-tile into four 1-bank psums."""
                pss = []
                for ut in range(KT):
                    colw = slice(base + ut * 128, base + (ut + 1) * 128)
                    ps = pp.tile([128, NB], dt.float32, tag="ps")
                    for t in range(KT):
                        nc.tensor.matmul(ps[:], lhs_a[:, t, colw], rhs_a[:, t, :],
                                         start=(t == 0), stop=False)
                    for t in range(KT):
                        nc.tensor.matmul(ps[:], lhs_b[:, t, colw], rhs_b[:, t, :],
                                         start=False, stop=(t == KT - 1))
                    pss.append(ps)
                return pss

            def tau_acts(pss, boff, tag):
                tau = wk.tile([128, KT, NB], dt.bfloat16, tag=tag)
                for ut in range(KT):
                    if specialized:
                        nc.scalar.activation(tau[:, ut, :], pss[ut][:], AF.Tanh,
                                             scale=0.5)
                    else:
                        nc.scalar.activation(tau[:, ut, :], pss[ut][:], AF.Tanh,
                                             bias=csb[:, ut, boff:boff + 1],
                                             scale=0.5)
                return tau

            # ---- prologue: chunk 0 loads + weights on the SWDGE queue ----
            tiles = load_chunk(0)
            for lo, hi, dst, src in (
                (0, F, usb, u_r), (0, F, wsb, w_r),
                (F, 2 * F, usb, u_r), (F, 2 * F, wsb, w_r),
                (2 * F, 3 * F, wsb, w_r), (2 * F, 3 * F, usb, u_r),
            ):
                nc.gpsimd.dma_start(out=dst[:, :, lo:hi], in_=src[:, :, lo:hi])
            gtiles = exp_chunk(tiles)
            hd, xd = preproc_chunk(tiles, gtiles)

            for c in range(NCHUNK):
                rows = slice(c * NB, (c + 1) * NB)
                last = c == NCHUNK - 1

                ps_z = gate_mms(0, usb, hd, wsb, xd)
                ps_r = gate_mms(F, usb, hd, wsb, xd)
                tau_z = tau_acts(ps_z, BZ, "tau_z")
                tau_r = tau_acts(ps_r, BR, "tau_r")

                if not last:
                    tiles_n = load_chunk(c + 1)
                    gtiles_n = exp_chunk(tiles_n)

                # rh' = (tau_r + 1) * hd   (U_h is host-scaled by 0.5)
                rh = wk.tile([128, KT, NB], dt.bfloat16, tag="rh")
                nc.vector.scalar_tensor_tensor(
                    rh[:], tau_r[:], 1.0, hd[:], OP.add, OP.mult)

                ps_h = gate_mms(2 * F, wsb, xd, usb, rh)

                hd_c, xd_c = hd, xd
                if not last:
                    hd, xd = preproc_chunk(tiles_n, gtiles_n)
                    tiles = tiles_n

                # h_hat = tanh(.+b_h); h_new = hd + (0.5 tau_z + 0.5)(hh-hd)
                for ut in range(KT):
                    hh = tp.tile([128, NB], dt.bfloat16, tag="hh")
                    if specialized:
                        nc.scalar.activation(hh[:], ps_h[ut][:], AF.Tanh)
                    else:
                        nc.scalar.activation(hh[:], ps_h[ut][:], AF.Tanh,
                                             bias=csb[:, ut, BH:BH + 1])
                    t6 = tp.tile([128, NB], dt.bfloat16, tag="t6")
                    nc.vector.tensor_sub(out=t6[:], in0=hh[:], in1=hd_c[:, ut, :])
                    t7 = tp.tile([128, NB], dt.bfloat16, tag="t7")
                    nc.vector.scalar_tensor_tensor(
                        t7[:], tau_z[:, ut, :], 1.0, t6[:], OP.add, OP.mult)
                    hn = tp.tile([128, NB], dt.bfloat16, tag="hn")
                    nc.vector.scalar_tensor_tensor(
                        hn[:], t7[:], 0.5, hd_c[:, ut, :], OP.mult, OP.add)
                    nc.sync.dma_start(out=o_r[:, ut, rows], in_=hn[:])

    nc.compile()
    return nc


def _get_nc(spec_key):
    if spec_key not in _STATE:
        _STATE[spec_key] = _build(spec_key)
    return _STATE[spec_key]


def _tp_cast(a):
    """[B, F] f32 view -> [F, B] contiguous bf16."""
    return np.ascontiguousarray(a.T).astype(BF16)


def kernel(**inputs) -> np.ndarray:
    from concourse import bass_utils

    inp = np.asarray(inputs["inputs"], dtype=np.float32)
    h_prev = np.asarray(inputs["h_prev"], dtype=np.float32)
    gx = np.maximum(np.asarray(inputs["gamma_x_decay"], np.float32), 0.0)
    gh = np.maximum(np.asarray(inputs["gamma_h_decay"], np.float32), 0.0)
    mi = np.asarray(inputs["mean_imputation"], np.float32)
    bz = np.asarray(inputs["b_z"], np.float32)
    br = np.asarray(inputs["b_r"], np.float32)
    bh = np.asarray(inputs["b_h"], np.float32)

    specialized = bool(
        np.all(gx == gx[0]) and np.all(gh == gx[0])
        and not np.any(mi) and not np.any(bz) and not np.any(br) and not np.any(bh)
    )
    spec_key = ("spec", float(-gx[0])) if specialized else "gen"
    nc = _get_nc(spec_key)

    xT = _tp_cast(inp[:, :F])
    mT = _tp_cast(inp[:, F:2 * F])
    dT = _tp_cast(inp[:, 2 * F:])
    hT = _tp_cast(h_prev)

    w = np.concatenate(
        [np.asarray(inputs["W_z"]), np.asarray(inputs["W_r"]), np.asarray(inputs["W_h"])],
        axis=1).astype(BF16)
    # fold the sigmoid-via-tanh 0.5 rescale of r into U_h
    u = np.concatenate(
        [np.asarray(inputs["U_z"]), np.asarray(inputs["U_r"]),
         0.5 * np.asarray(inputs["U_h"])],
        axis=1).astype(BF16)

    in_maps = []
    for c in range(N_CORES):
        cols = slice(c * BC, (c + 1) * BC)
        im = {"x": xT[:, cols], "m": mT[:, cols], "d": dT[:, cols], "h": hT[:, cols],
              "w": w, "u": u}
        if not specialized:
            # half-biases for z/r: tanh((pre+b)/2) takes b/2 as the ACT bias
            consts = np.stack([-gx, -gh, mi, 0.5 * bz, 0.5 * br, bh], axis=-1)
            im["c"] = np.ascontiguousarray(
                consts.reshape(KT, 128, 6).transpose(1, 0, 2))
        in_maps.append(im)

    res = bass_utils.run_bass_kernel_spmd(
        nc, in_maps, core_ids=list(range(N_CORES)), **_STATE.get("run_kwargs", {})
    )
    _STATE["last_results"] = res

    out = np.empty((B, F), np.float32)
    for c in range(N_CORES):
        out[c * BC:(c + 1) * BC, :] = res.results[c]["o"].T.astype(np.float32)
    return out
